# revision 6
# baseline (speedup 1.0000x reference)
"""AtomPlacementScheduler Trainium2 kernel (v4: bf16 + engine balance).

out[b] = sum_e irfft(rfft(stems[b,e]) * exp(-2i pi f s_be)),  s = sigmoid(TL@W+b)*N.

4-step FFT, half-spectrum form: the full signed-frequency grid
k~ = k2 + 256*k1 with k2 in [0,128] (129 cols, padded to 132) and SIGNED
k1 in [-64,63] (128 rows) covers every conjugate pair of the real-signal
spectrum exactly once (k2 in {0,128} columns are self-paired, weight 1;
k2 in [1,127] carry weight 2 + real part).  Shift phase factors exactly as
A[k2]*B[k1] on this grid (no partial-row corrections), so per event the
device does: 2 stage-1 matmuls (264 free), 1 PSUM->SBUF copy, 6 half-width
elementwise ops (DVE re-chain, GpSimd im-chain), 2 stage-3 matmuls (264
free) accumulating the event sum in PSUM.  All twiddle tables (C = T*A_e,
M = W1*B_e) are host-precomputed and DMA'd fp16 in one fused transfer.
The inverse (per batch) is I1 -> twiddle*d/N -> transpose -> I4, exact
(no host correction).

Self-contained: hardcodes shapes B=64, E=16, N=32768, n_cores=8.
"""
import numpy as np
import ml_dtypes

N = 32768
N1 = 128
N2 = 256
E = 16
B = 64
NCORES = 8
BC = B // NCORES
K2 = 129            # k2 = 0..128
KP = 132            # padded k2 width
F32 = np.float32
F16 = ml_dtypes.bfloat16
GSC = np.float32(1.0 / 16.0)


def _host_tables():
    n1 = np.arange(N1)
    n2 = np.arange(N2)
    k2 = np.arange(K2)
    kap = np.arange(N1) - 64                       # signed k1
    W2 = np.exp(-2j * np.pi * np.outer(n2, k2) / N2)        # (256, 129)
    T = np.exp(-2j * np.pi * np.outer(n1, k2) / N)          # (128, 129)
    W1s = np.exp(-2j * np.pi * np.outer(n1, kap) / N1)      # (128, 128)
    E1s = np.exp(+2j * np.pi * np.outer(kap, n1) / N1) * GSC  # (128, 128) [j, n1]
    d = np.where((k2 == 0) | (k2 == 128), 1.0, 2.0)
    TW = np.exp(+2j * np.pi * np.outer(n1, k2) / N) * (d / (N * GSC))  # (128,129)
    E2 = np.exp(+2j * np.pi * np.outer(np.arange(K2), n2) / N2)        # (129, 256)
    return W2, T, W1s, E1s, TW, E2


def _pad(a, w=KP):
    # pad last axis to w with zeros
    out = np.zeros(a.shape[:-1] + (w,), dtype=a.dtype)
    out[..., : a.shape[-1]] = a
    return out


def _build_graph():
    import concourse.bass as bass
    import concourse.mybir as mybir
    import concourse.tile as tile
    from concourse import bacc

    dt = mybir.dt
    nc = bacc.Bacc("TRN2", target_bir_lowering=False, debug=False, num_devices=NCORES)

    W = 2 * KP          # 264: [re | im]
    stems_d = nc.dram_tensor("stems16", [BC, E, N1, N2], dt.bfloat16, kind="ExternalInput")
    cm_d = nc.dram_tensor("cm_tab", [BC, E, N1, W + 256], dt.bfloat16, kind="ExternalInput")
    w2_d = nc.dram_tensor("w2cat", [N2, W], dt.bfloat16, kind="ExternalInput")
    e1c_d = nc.dram_tensor("e1sc", [N1, N1], dt.bfloat16, kind="ExternalInput")
    e1s_d = nc.dram_tensor("e1ss", [N1, N1], dt.bfloat16, kind="ExternalInput")
    twc_d = nc.dram_tensor("twc", [N1, KP], dt.bfloat16, kind="ExternalInput")
    tws_d = nc.dram_tensor("tws", [N1, KP], dt.bfloat16, kind="ExternalInput")
    e2c0_d = nc.dram_tensor("e2c0", [128, N2], dt.bfloat16, kind="ExternalInput")
    e2sn0_d = nc.dram_tensor("e2sn0", [128, N2], dt.bfloat16, kind="ExternalInput")
    e2c1_d = nc.dram_tensor("e2c1", [128, N2], dt.bfloat16, kind="ExternalInput")
    out_d = nc.dram_tensor("out", [BC, N2, N1], dt.float32, kind="ExternalOutput")

    with tile.TileContext(nc) as tc:
        with (
            tc.tile_pool(name="const", bufs=1) as cpool,
            tc.tile_pool(name="work", bufs=6) as pool,
            tc.tile_pool(name="binv", bufs=2) as bpool,
            tc.tile_pool(name="psum", bufs=3, space="PSUM") as psum,
            tc.tile_pool(name="psacc", bufs=1, space="PSUM") as psacc,
            tc.tile_pool(name="pinv", bufs=1, space="PSUM") as pinv,
        ):
            w2h0 = cpool.tile([128, W], dt.bfloat16, tag="w2h0")
            w2h1 = cpool.tile([128, W], dt.bfloat16, tag="w2h1")
            nc.sync.dma_start(w2h0[:], w2_d[0:128, :])
            nc.sync.dma_start(w2h1[:], w2_d[128:256, :])
            e1sc = cpool.tile([N1, N1], dt.bfloat16, tag="e1sc")
            e1ss = cpool.tile([N1, N1], dt.bfloat16, tag="e1ss")
            nc.sync.dma_start(e1sc[:], e1c_d[:])
            nc.sync.dma_start(e1ss[:], e1s_d[:])
            twc = cpool.tile([N1, KP], dt.bfloat16, tag="twc")
            tws = cpool.tile([N1, KP], dt.bfloat16, tag="tws")
            nc.sync.dma_start(twc[:], twc_d[:])
            nc.sync.dma_start(tws[:], tws_d[:])
            e2c0 = cpool.tile([128, N2], dt.bfloat16, tag="e2c0")
            e2sn0 = cpool.tile([128, N2], dt.bfloat16, tag="e2sn0")
            e2c1 = cpool.tile([128, N2], dt.bfloat16, tag="e2c1")
            nc.sync.dma_start(e2c0[:], e2c0_d[:])
            nc.sync.dma_start(e2sn0[:], e2sn0_d[:])
            nc.sync.dma_start(e2c1[:], e2c1_d[:])

            for b in range(BC):
                pZA = psacc.tile([N1, W], dt.float32, tag="pZA")
                pZB = psacc.tile([N1, W], dt.float32, tag="pZB")
                for e in range(E):
                    xm = pool.tile([128, N2], dt.bfloat16, tag="xm")
                    nc.scalar.dma_start(xm[:], stems_d[b, e])
                    cm = pool.tile([N1, W + 256], dt.bfloat16, tag="cm")
                    nc.scalar.dma_start(cm[:], cm_d[b, e])
                    p1 = psum.tile([N1, W], dt.float32, tag="p1")
                    nc.tensor.matmul(p1[:], xm[:, 0:128], w2h0[:], start=True, stop=False)
                    nc.tensor.matmul(p1[:], xm[:, 128:256], w2h1[:], start=False, stop=True)
                    p1sb = pool.tile([N1, W], dt.bfloat16, tag="p1sb")
                    nc.scalar.copy(p1sb[:], p1[:])
                    # U = P1 * C  (C = cm[:, 0:264]); re on DVE, im on GpSimd
                    uv = pool.tile([N1, W], dt.bfloat16, tag="uv")
                    t1 = pool.tile([N1, KP], dt.bfloat16, tag="t1")
                    t2 = pool.tile([N1, KP], dt.bfloat16, tag="t2")
                    t3 = pool.tile([N1, KP], dt.bfloat16, tag="t3")
                    t4 = pool.tile([N1, KP], dt.bfloat16, tag="t4")
                    nc.vector.tensor_mul(t1[:], p1sb[:, 0:KP], cm[:, 0:KP])
                    nc.vector.tensor_mul(t2[:], p1sb[:, KP:W], cm[:, KP:W])
                    nc.vector.tensor_sub(uv[:, 0:KP], t1[:], t2[:])
                    nc.vector.tensor_mul(t3[:], p1sb[:, 0:KP], cm[:, KP:W])
                    nc.gpsimd.tensor_mul(t4[:], p1sb[:, KP:W], cm[:, 0:KP])
                    nc.vector.tensor_add(uv[:, KP:W], t3[:], t4[:])
                    # stage 3: accumulate over events; M_re/M_im from cm tail
                    nc.tensor.matmul(pZA[:], cm[:, W : W + 128], uv[:],
                                     start=(e == 0), stop=(e == E - 1))
                    nc.tensor.matmul(pZB[:], cm[:, W + 128 : W + 256], uv[:],
                                     start=(e == 0), stop=(e == E - 1))
                # xf = Z (128, 264)
                xf = bpool.tile([N1, W], dt.bfloat16, tag="xf")
                pbsb = bpool.tile([N1, W], dt.bfloat16, tag="pbsb")
                nc.scalar.copy(pbsb[:], pZB[:])
                nc.any.tensor_sub(xf[:, 0:KP], pZA[:, 0:KP], pbsb[:, KP:W])
                nc.any.tensor_add(xf[:, KP:W], pZA[:, KP:W], pbsb[:, 0:KP])
                # I1: G = E1s^T @ Z
                pga = pinv.tile([N1, W], dt.float32, tag="pga")
                pgb = pinv.tile([N1, W], dt.float32, tag="pgb")
                nc.tensor.matmul(pga[:], e1sc[:], xf[:], start=True, stop=True)
                nc.tensor.matmul(pgb[:], e1ss[:], xf[:], start=True, stop=True)
                g_re = bpool.tile([N1, KP], dt.bfloat16, tag="gre")
                g_im = bpool.tile([N1, KP], dt.bfloat16, tag="gim")
                gbsb = bpool.tile([N1, W], dt.bfloat16, tag="gbsb")
                nc.scalar.copy(gbsb[:], pgb[:])
                nc.any.tensor_sub(g_re[:], pga[:, 0:KP], gbsb[:, KP:W])
                nc.any.tensor_add(g_im[:], pga[:, KP:W], gbsb[:, 0:KP])
                # GT = G * TW  (d/N folded in); gt_re padded to 256 for transpose
                gt_re = bpool.tile([N1, N2], dt.bfloat16, tag="gtre")
                gt_im = bpool.tile([N1, KP], dt.bfloat16, tag="gtim")
                i1 = bpool.tile([N1, KP], dt.bfloat16, tag="i1")
                i2 = bpool.tile([N1, KP], dt.bfloat16, tag="i2")
                nc.vector.tensor_mul(i1[:], g_re[:], twc[:])
                nc.vector.tensor_mul(i2[:], g_im[:], tws[:])
                nc.vector.tensor_sub(gt_re[:, 0:KP], i1[:], i2[:])
                nc.gpsimd.tensor_mul(i1[:], g_re[:], tws[:])
                nc.gpsimd.tensor_mul(i2[:], g_im[:], twc[:])
                nc.gpsimd.tensor_add(gt_im[:], i1[:], i2[:])
                # transposes: (k2, n1) chunks
                gttre0 = bpool.tile([128, N1], dt.bfloat16, tag="gttre0")
                gttre1 = bpool.tile([128, N1], dt.bfloat16, tag="gttre1")
                gttim0 = bpool.tile([128, N1], dt.bfloat16, tag="gttim0")
                nc.sync.dma_start_transpose(gttre0[:], gt_re[:, 0:128])
                nc.sync.dma_start_transpose(gttre1[:], gt_re[:, 128:256])
                nc.sync.dma_start_transpose(gttim0[:], gt_im[:, 0:128])
                # I4: y[n2, n1] = sum_k2 Re(E2 * GT^T)
                for jc in range(2):
                    js = slice(128 * jc, 128 * jc + 128)
                    ps = pinv.tile([128, N1], dt.float32, tag="ps")
                    nc.tensor.matmul(ps[:], e2c0[:, js], gttre0[:], start=True, stop=False)
                    nc.tensor.matmul(ps[:], e2sn0[:, js], gttim0[:], start=False, stop=False)
                    nc.tensor.matmul(ps[:], e2c1[:, js], gttre1[:], start=False, stop=True)
                    y_sb = bpool.tile([128, N1], dt.float32, tag="ysb")
                    nc.scalar.copy(y_sb[:], ps[:])
                    nc.sync.dma_start(out_d[b, js, :], y_sb[:])
    nc.compile()
    return nc


def kernel(time_latent, stems, targets, W_pos, b_pos):
    from concourse.bass_utils import run_bass_kernel_spmd

    z = np.einsum("bed,od->beo", time_latent.astype(F32), W_pos.astype(F32))
    z = z.reshape(B, E) + b_pos.reshape(1)[0]
    pos = 1.0 / (1.0 + np.exp(-z, dtype=F32))
    s = pos * np.float32(N)

    W2, T, W1s, E1s, TW, E2 = _host_tables()
    k2 = np.arange(K2)
    kap = np.arange(N1) - 64

    # stems: (B,E,32768) -> (B,E,128,256) fp16, cols [n2<128 | n2>=128]
    x = stems.reshape(B, E, N2, N1).astype(F16)
    x = x.reshape(B, E, 2, 128, N1).transpose(0, 1, 3, 2, 4).reshape(B, E, N1, N2)

    w2cat = np.concatenate([_pad(W2.real), _pad(W2.imag)], 1)  # (256, 264)

    nc = _build_graph()
    in_maps = []
    for c in range(NCORES):
        sl = slice(c * BC, (c + 1) * BC)
        s_c = s[sl].astype(np.float64)                        # (BC, E)
        A = np.exp(-2j * np.pi * s_c[..., None] * k2 / N)     # (BC,E,129)
        Bs = np.exp(-2j * np.pi * s_c[..., None] * kap / N1)  # (BC,E,128)
        C = T[None, None] * A[:, :, None, :]                  # (BC,E,128,129)
        M = W1s[None, None] * Bs[:, :, None, :]               # (BC,E,128,128)
        cm = np.concatenate(
            [_pad(C.real), _pad(C.imag), M.real, M.imag], -1).astype(F16)
        in_maps.append({
            "stems16": np.ascontiguousarray(x[sl]),
            "cm_tab": cm,                                     # (BC,E,128,520)
            "w2cat": w2cat.astype(F16),
            "e1sc": E1s.real.astype(F16),
            "e1ss": E1s.imag.astype(F16),
            "twc": _pad(TW.real).astype(F16),
            "tws": _pad(TW.imag).astype(F16),
            "e2c0": E2.real[0:128].astype(F16),
            "e2sn0": (-E2.imag[0:128]).astype(F16),
            "e2c1": np.concatenate([E2.real[128:129], np.zeros((127, N2))], 0).astype(F16),
        })

    import os
    trace = bool(int(os.environ.get("ATHENA_TRACE", "0")))
    res = run_bass_kernel_spmd(nc, in_maps, core_ids=list(range(NCORES)), trace=trace)
    if trace:
        print(f"HW exec time: {res.exec_time_ns} ns")
    outs = [res.results[c]["out"].reshape(BC, N).astype(F32) for c in range(NCORES)]
    return np.concatenate(outs, 0).reshape(B, 1, N).astype(F32)


# revision 10
# speedup vs baseline: 1.4291x; 1.4291x over previous
"""AtomPlacementScheduler Trainium2 kernel (v5).

out[b] = sum_e irfft(rfft(stems[b,e]) * exp(-2i pi f s_be)),  s = sigmoid(TL@W+b)*N.

4-step FFT on the signed-frequency half grid k~ = k2 + 256*k1, k2 in [0,128],
k1 signed in [-64,63]; every conjugate pair of the real-signal spectrum is
covered exactly once (k2 in {0,128} self-paired weight 1, k2 in [1,127]
weight 2 + real part).  Column layout keeps the main k2 = 1..128 block
exactly 128 wide (64B-aligned slices -> DVE 2x mode); the DC column (k2=0,
where all phases are 1) rides along via 1-column matmuls.  Per-event device
work: 2 stage-1 matmuls, 1 PSUM->SBUF copy, 6 x 128-wide elementwise ops
(split DVE/GpSimd, alternating), 2+2 stage-3 matmuls accumulating the event
sum in PSUM.  C = T*A_e and M = W1*B_e are host-precomputed, one fused bf16
DMA per event.  The inverse runs transposed (stationary = Z chunks) so no
DMA transposes are needed; it is exact (no host correction).

Self-contained: hardcodes shapes B=64, E=16, N=32768, n_cores=8.
"""
import numpy as np
import ml_dtypes

N = 32768
N1 = 128
N2 = 256
E = 16
B = 64
NCORES = 8
BC = B // NCORES
F32 = np.float32
BF16 = ml_dtypes.bfloat16
GSC = np.float32(1.0 / 16.0)
WMAIN = 256          # [re(k2=1..128) | im(k2=1..128)]
WS = 258             # + [dc-re | dc-im]
CMW = WS + 256       # c section + m section


def _host_tables():
    n1 = np.arange(N1)
    n2 = np.arange(N2)
    k2m = np.arange(1, 129)
    kap = np.arange(N1) - 64
    W2m = np.exp(-2j * np.pi * np.outer(n2, k2m) / N2)       # (256,128)
    Tm = np.exp(-2j * np.pi * np.outer(n1, k2m) / N)         # (128,128)
    W1s = np.exp(-2j * np.pi * np.outer(n1, kap) / N1)       # (128,128)
    E1 = np.exp(+2j * np.pi * np.outer(kap, n1) / N1) * GSC  # (j,n1)
    d = np.where(k2m == 128, 1.0, 2.0)
    TWt = np.exp(+2j * np.pi * np.outer(k2m, n1) / N) * (d[:, None] / (N * GSC))
    E2m = np.exp(+2j * np.pi * np.outer(k2m, n2) / N2)       # (128,256)
    return W2m, Tm, W1s, E1, TWt, E2m


def _build_graph():
    import concourse.bass as bass
    import concourse.mybir as mybir
    import concourse.tile as tile
    from concourse import bacc

    dt = mybir.dt
    nc = bacc.Bacc("TRN2", target_bir_lowering=False, debug=False, num_devices=NCORES)

    stems_d = nc.dram_tensor("stems16", [BC, E, N1, N2], dt.bfloat16, kind="ExternalInput")
    cm_d = nc.dram_tensor("cm_tab", [BC, E, N1, CMW], dt.bfloat16, kind="ExternalInput")
    w2_d = nc.dram_tensor("w2cat", [N2, WS], dt.bfloat16, kind="ExternalInput")
    e1_d = nc.dram_tensor("e1cat", [N1, 384], dt.bfloat16, kind="ExternalInput")
    twc_d = nc.dram_tensor("twtc", [N1, N1], dt.bfloat16, kind="ExternalInput")
    tws_d = nc.dram_tensor("twts", [N1, N1], dt.bfloat16, kind="ExternalInput")
    e2c0_d = nc.dram_tensor("e2c0", [128, N2], dt.bfloat16, kind="ExternalInput")
    e2sn0_d = nc.dram_tensor("e2sn0", [128, N2], dt.bfloat16, kind="ExternalInput")
    e2c1_d = nc.dram_tensor("e2c1", [128, N2], dt.bfloat16, kind="ExternalInput")
    out_d = nc.dram_tensor("out", [BC, N2, N1], dt.float32, kind="ExternalOutput")

    with tile.TileContext(nc) as tc:
        with (
            tc.tile_pool(name="const", bufs=1) as cpool,
            tc.tile_pool(name="work", bufs=6) as pool,
            tc.tile_pool(name="binv", bufs=2) as bpool,
            tc.tile_pool(name="psum", bufs=2, space="PSUM") as psum,
            tc.tile_pool(name="psacc", bufs=1, space="PSUM") as psacc,
            tc.tile_pool(name="pinv", bufs=1, space="PSUM") as pinv,
        ):
            w2h0 = cpool.tile([128, WS], dt.bfloat16, tag="w2h0")
            w2h1 = cpool.tile([128, WS], dt.bfloat16, tag="w2h1")
            nc.sync.dma_start(w2h0[:], w2_d[0:128, :])
            nc.sync.dma_start(w2h1[:], w2_d[128:256, :])
            e1cat = cpool.tile([N1, 384], dt.bfloat16, tag="e1cat")
            nc.sync.dma_start(e1cat[:], e1_d[:])
            twtc = cpool.tile([N1, N1], dt.bfloat16, tag="twtc")
            twts = cpool.tile([N1, N1], dt.bfloat16, tag="twts")
            nc.sync.dma_start(twtc[:], twc_d[:])
            nc.sync.dma_start(twts[:], tws_d[:])
            e2c0 = cpool.tile([128, N2], dt.bfloat16, tag="e2c0")
            e2sn0 = cpool.tile([128, N2], dt.bfloat16, tag="e2sn0")
            e2c1 = cpool.tile([128, N2], dt.bfloat16, tag="e2c1")
            nc.sync.dma_start(e2c0[:], e2c0_d[:])
            nc.sync.dma_start(e2sn0[:], e2sn0_d[:])
            nc.sync.dma_start(e2c1[:], e2c1_d[:])
            gtdc = cpool.tile([128, N1], dt.bfloat16, tag="gtdc")
            nc.vector.memset(gtdc[:], 0.0)

            for b in range(BC):
                pZA = psacc.tile([N1, WS], dt.float32, tag="pZA")
                pZB = psacc.tile([N1, WS], dt.float32, tag="pZB")
                for e in range(E):
                    xm = pool.tile([128, N2], dt.bfloat16, tag="xm")
                    nc.sync.dma_start(xm[:], stems_d[b, e])
                    cm = pool.tile([N1, CMW], dt.bfloat16, tag="cm")
                    nc.sync.dma_start(cm[:], cm_d[b, e])
                    p1 = psum.tile([N1, WS], dt.float32, tag="p1")
                    nc.tensor.matmul(p1[:], xm[:, 0:128], w2h0[:], start=True, stop=False)
                    nc.tensor.matmul(p1[:], xm[:, 128:256], w2h1[:], start=False, stop=True)
                    p1sb = pool.tile([N1, WS], dt.bfloat16, tag="p1sb")
                    nc.scalar.copy(p1sb[:], p1[:])
                    # U = P1 * C  (all slices 128-wide, 64B-aligned -> 2x mode)
                    uv = pool.tile([N1, WS], dt.bfloat16, tag="uv")
                    t1 = pool.tile([N1, 128], dt.bfloat16, tag="t1")
                    t2 = pool.tile([N1, 128], dt.bfloat16, tag="t2")
                    t3 = pool.tile([N1, 128], dt.bfloat16, tag="t3")
                    t4 = pool.tile([N1, 128], dt.bfloat16, tag="t4")
                    nc.vector.tensor_mul(t1[:], p1sb[:, 0:128], cm[:, 0:128])
                    nc.vector.tensor_mul(t2[:], p1sb[:, 128:256], cm[:, 128:256])
                    nc.vector.tensor_sub(uv[:, 0:128], t1[:], t2[:])
                    if e % 2 == 0:
                        nc.vector.tensor_mul(t3[:], p1sb[:, 0:128], cm[:, 128:256])
                        nc.gpsimd.tensor_mul(t4[:], p1sb[:, 128:256], cm[:, 0:128])
                    else:
                        nc.gpsimd.tensor_mul(t3[:], p1sb[:, 0:128], cm[:, 128:256])
                        nc.gpsimd.tensor_mul(t4[:], p1sb[:, 128:256], cm[:, 0:128])
                    nc.vector.tensor_add(uv[:, 128:256], t3[:], t4[:])
                    nc.vector.tensor_copy(uv[:, 256:257], p1sb[:, 256:257])
                    # stage 3: one accumulation group per PSUM bank (incl. DC col)
                    nc.tensor.matmul(pZA[:, 0:257], cm[:, WS : WS + 128], uv[:, 0:257],
                                     start=(e == 0), stop=(e == E - 1))
                    nc.tensor.matmul(pZB[:, 0:257], cm[:, WS + 128 : WS + 256], uv[:, 0:257],
                                     start=(e == 0), stop=(e == E - 1))
                # xf = Z (128, 258): [re-main | im-main | dc-re | dc-im]
                xf = bpool.tile([N1, WS], dt.bfloat16, tag="xf")
                pbsb = bpool.tile([N1, WS], dt.bfloat16, tag="pbsb")
                nc.scalar.copy(pbsb[:], pZB[:])
                nc.any.tensor_sub(xf[:, 0:128], pZA[:, 0:128], pbsb[:, 128:256])
                nc.any.tensor_add(xf[:, 128:256], pZA[:, 128:256], pbsb[:, 0:128])
                nc.any.tensor_copy(xf[:, 256:257], pZA[:, 256:257])
                nc.any.tensor_copy(xf[:, 257:258], pbsb[:, 256:257])
                # I1 transposed: G^T chunks = xf_chunk^T @ [E1c | E1s]
                pgA = pinv.tile([N1, 256], dt.float32, tag="pgA")
                pgB = pinv.tile([N1, 256], dt.float32, tag="pgB")
                psdc = pinv.tile([1, 128], dt.float32, tag="psdc")
                nc.tensor.matmul(pgA[:], xf[:, 0:128], e1cat[:, 0:256], start=True, stop=True)
                nc.tensor.matmul(pgB[:], xf[:, 128:256], e1cat[:, 0:256], start=True, stop=True)
                nc.tensor.matmul(psdc[:], xf[:, 256:257], e1cat[:, 0:128], start=True, stop=False)
                nc.tensor.matmul(psdc[:], xf[:, 257:258], e1cat[:, 256:384], start=False, stop=True)
                gbsb = bpool.tile([N1, 256], dt.bfloat16, tag="gbsb")
                nc.scalar.copy(gbsb[:], pgB[:])
                g_re = bpool.tile([N1, N1], dt.bfloat16, tag="gre")
                g_im = bpool.tile([N1, N1], dt.bfloat16, tag="gim")
                nc.any.tensor_sub(g_re[:], pgA[:, 0:128], gbsb[:, 128:256])
                nc.any.tensor_add(g_im[:], pgA[:, 128:256], gbsb[:, 0:128])
                # twiddle (transposed layout, d/N folded)
                gttre = bpool.tile([N1, N1], dt.bfloat16, tag="gttre")
                gttim = bpool.tile([N1, N1], dt.bfloat16, tag="gttim")
                i1 = bpool.tile([N1, N1], dt.bfloat16, tag="i1")
                i2 = bpool.tile([N1, N1], dt.bfloat16, tag="i2")
                nc.any.tensor_mul(i1[:], g_re[:], twtc[:])
                nc.any.tensor_mul(i2[:], g_im[:], twts[:])
                nc.any.tensor_sub(gttre[:], i1[:], i2[:])
                nc.any.tensor_mul(i1[:], g_re[:], twts[:])
                nc.any.tensor_mul(i2[:], g_im[:], twtc[:])
                nc.any.tensor_add(gttim[:], i1[:], i2[:])
                # DC row -> row 0 of gtdc (TW const folded into e2c1)
                nc.any.tensor_copy(gtdc[0:1, :], psdc[0:1, :])
                # I4
                for jc in range(2):
                    js = slice(128 * jc, 128 * jc + 128)
                    ps = pinv.tile([128, N1], dt.float32, tag="ps")
                    nc.tensor.matmul(ps[:], e2c0[:, js], gttre[:], start=True, stop=False)
                    nc.tensor.matmul(ps[:], e2sn0[:, js], gttim[:], start=False, stop=False)
                    nc.tensor.matmul(ps[:], e2c1[:, js], gtdc[:], start=False, stop=True)
                    y_sb = bpool.tile([128, N1], dt.float32, tag="ysb")
                    nc.scalar.copy(y_sb[:], ps[:])
                    nc.scalar.dma_start(out_d[b, js, :], y_sb[:])
    nc.compile()
    return nc


def kernel(time_latent, stems, targets, W_pos, b_pos):
    from concourse.bass_utils import run_bass_kernel_spmd

    z = np.einsum("bed,od->beo", time_latent.astype(F32), W_pos.astype(F32))
    z = z.reshape(B, E) + b_pos.reshape(1)[0]
    pos = 1.0 / (1.0 + np.exp(-z, dtype=F32))
    s = pos * np.float32(N)

    W2m, Tm, W1s, E1, TWt, E2m = _host_tables()
    k2m = np.arange(1, 129)
    kap = np.arange(N1) - 64

    # stems: (B,E,32768) -> (B,E,128,256) bf16, cols [n2<128 | n2>=128]
    x = stems.reshape(B, E, N2, N1).astype(BF16)
    x = x.reshape(B, E, 2, 128, N1).transpose(0, 1, 3, 2, 4).reshape(B, E, N1, N2)

    w2cat = np.concatenate(
        [W2m.real, W2m.imag, np.ones((N2, 1)), np.zeros((N2, 1))], 1)  # (256,258)

    nc = _build_graph()
    in_maps = []
    for c in range(NCORES):
        sl = slice(c * BC, (c + 1) * BC)
        s_c = s[sl].astype(np.float64)                          # (BC, E)
        A = np.exp(-2j * np.pi * s_c[..., None] * k2m / N)      # (BC,E,128)
        Bs = np.exp(-2j * np.pi * s_c[..., None] * kap / N1)    # (BC,E,128)
        C = Tm[None, None] * A[:, :, None, :]                   # (BC,E,128,128)
        M = W1s[None, None] * Bs[:, :, None, :]                 # (BC,E,128,128)
        zc = np.zeros(C.shape[:-1] + (1,))
        cm = np.concatenate([C.real, C.imag, zc, zc, M.real, M.imag], -1).astype(BF16)
        in_maps.append({
            "stems16": np.ascontiguousarray(x[sl]),
            "cm_tab": cm,                                       # (BC,E,128,514)
            "w2cat": w2cat.astype(BF16),
            "e1cat": np.concatenate([E1.real, E1.imag, -E1.imag], 1).astype(BF16),
            "twtc": TWt.real.astype(BF16),
            "twts": TWt.imag.astype(BF16),
            "e2c0": E2m.real.astype(BF16),
            "e2sn0": (-E2m.imag).astype(BF16),
            "e2c1": np.concatenate(
                [np.full((1, N2), 1.0 / (N * GSC)), np.zeros((127, N2))], 0).astype(BF16),
        })

    import os
    trace = bool(int(os.environ.get("ATHENA_TRACE", "0")))
    res = run_bass_kernel_spmd(nc, in_maps, core_ids=list(range(NCORES)), trace=trace)
    if trace:
        print(f"HW exec time: {res.exec_time_ns} ns")
    outs = [res.results[c]["out"].reshape(BC, N).astype(F32) for c in range(NCORES)]
    return np.concatenate(outs, 0).reshape(B, 1, N).astype(F32)


# revision 11
# speedup vs baseline: 1.4706x; 1.0291x over previous
"""AtomPlacementScheduler Trainium2 kernel (v5).

out[b] = sum_e irfft(rfft(stems[b,e]) * exp(-2i pi f s_be)),  s = sigmoid(TL@W+b)*N.

4-step FFT on the signed-frequency half grid k~ = k2 + 256*k1, k2 in [0,128],
k1 signed in [-64,63]; every conjugate pair of the real-signal spectrum is
covered exactly once (k2 in {0,128} self-paired weight 1, k2 in [1,127]
weight 2 + real part).  Column layout keeps the main k2 = 1..128 block
exactly 128 wide (64B-aligned slices -> DVE 2x mode); the DC column (k2=0,
where all phases are 1) rides along via 1-column matmuls.  Per-event device
work: 2 stage-1 matmuls, 1 PSUM->SBUF copy, 6 x 128-wide elementwise ops
(split DVE/GpSimd, alternating), 2+2 stage-3 matmuls accumulating the event
sum in PSUM.  C = T*A_e and M = W1*B_e are host-precomputed, one fused bf16
DMA per event.  The inverse runs transposed (stationary = Z chunks) so no
DMA transposes are needed; it is exact (no host correction).

Self-contained: hardcodes shapes B=64, E=16, N=32768, n_cores=8.
"""
import numpy as np
import ml_dtypes

N = 32768
N1 = 128
N2 = 256
E = 16
B = 64
NCORES = 8
BC = B // NCORES
F32 = np.float32
BF16 = ml_dtypes.bfloat16
GSC = np.float32(1.0 / 16.0)
WMAIN = 256          # [re(k2=1..128) | im(k2=1..128)]
WS = 258             # + [dc-re | dc-im]
CMW = WS + 256       # c section + m section


def _host_tables():
    n1 = np.arange(N1)
    n2 = np.arange(N2)
    k2m = np.arange(1, 129)
    kap = np.arange(N1) - 64
    W2m = np.exp(-2j * np.pi * np.outer(n2, k2m) / N2)       # (256,128)
    Tm = np.exp(-2j * np.pi * np.outer(n1, k2m) / N)         # (128,128)
    W1s = np.exp(-2j * np.pi * np.outer(n1, kap) / N1)       # (128,128)
    E1 = np.exp(+2j * np.pi * np.outer(kap, n1) / N1) * GSC  # (j,n1)
    d = np.where(k2m == 128, 1.0, 2.0)
    TWt = np.exp(+2j * np.pi * np.outer(k2m, n1) / N) * (d[:, None] / (N * GSC))
    E2m = np.exp(+2j * np.pi * np.outer(k2m, n2) / N2)       # (128,256)
    return W2m, Tm, W1s, E1, TWt, E2m


def _build_graph():
    import concourse.bass as bass
    import concourse.mybir as mybir
    import concourse.tile as tile
    from concourse import bacc

    dt = mybir.dt
    nc = bacc.Bacc("TRN2", target_bir_lowering=False, debug=False, num_devices=NCORES)

    xc_d = nc.dram_tensor("xmcm", [BC, E, N1, N2 + CMW], dt.bfloat16, kind="ExternalInput")
    w2_d = nc.dram_tensor("w2cat", [N2, WS], dt.bfloat16, kind="ExternalInput")
    e1_d = nc.dram_tensor("e1cat", [N1, 384], dt.bfloat16, kind="ExternalInput")
    twc_d = nc.dram_tensor("twtc", [N1, N1], dt.bfloat16, kind="ExternalInput")
    tws_d = nc.dram_tensor("twts", [N1, N1], dt.bfloat16, kind="ExternalInput")
    e2c0_d = nc.dram_tensor("e2c0", [128, N2], dt.bfloat16, kind="ExternalInput")
    e2sn0_d = nc.dram_tensor("e2sn0", [128, N2], dt.bfloat16, kind="ExternalInput")
    e2c1_d = nc.dram_tensor("e2c1", [128, N2], dt.bfloat16, kind="ExternalInput")
    out_d = nc.dram_tensor("out", [BC, N2, N1], dt.float32, kind="ExternalOutput")

    with tile.TileContext(nc) as tc:
        with (
            tc.tile_pool(name="const", bufs=1) as cpool,
            tc.tile_pool(name="work", bufs=6) as pool,
            tc.tile_pool(name="binv", bufs=2) as bpool,
            tc.tile_pool(name="psum", bufs=2, space="PSUM") as psum,
            tc.tile_pool(name="psacc", bufs=1, space="PSUM") as psacc,
            tc.tile_pool(name="pinv", bufs=1, space="PSUM") as pinv,
        ):
            w2h0 = cpool.tile([128, WS], dt.bfloat16, tag="w2h0")
            w2h1 = cpool.tile([128, WS], dt.bfloat16, tag="w2h1")
            nc.sync.dma_start(w2h0[:], w2_d[0:128, :])
            nc.sync.dma_start(w2h1[:], w2_d[128:256, :])
            e1cat = cpool.tile([N1, 384], dt.bfloat16, tag="e1cat")
            nc.sync.dma_start(e1cat[:], e1_d[:])
            twtc = cpool.tile([N1, N1], dt.bfloat16, tag="twtc")
            twts = cpool.tile([N1, N1], dt.bfloat16, tag="twts")
            nc.sync.dma_start(twtc[:], twc_d[:])
            nc.sync.dma_start(twts[:], tws_d[:])
            e2c0 = cpool.tile([128, N2], dt.bfloat16, tag="e2c0")
            e2sn0 = cpool.tile([128, N2], dt.bfloat16, tag="e2sn0")
            e2c1 = cpool.tile([128, N2], dt.bfloat16, tag="e2c1")
            nc.sync.dma_start(e2c0[:], e2c0_d[:])
            nc.sync.dma_start(e2sn0[:], e2sn0_d[:])
            nc.sync.dma_start(e2c1[:], e2c1_d[:])
            gtdc = cpool.tile([128, N1], dt.bfloat16, tag="gtdc")
            nc.vector.memset(gtdc[:], 0.0)

            for b in range(BC):
                pZA = psacc.tile([N1, WS], dt.float32, tag="pZA")
                pZB = psacc.tile([N1, WS], dt.float32, tag="pZB")
                for e in range(E):
                    xc = pool.tile([128, N2 + CMW], dt.bfloat16, tag="xc")
                    nc.sync.dma_start(xc[:], xc_d[b, e])
                    p1 = psum.tile([N1, WS], dt.float32, tag="p1")
                    nc.tensor.matmul(p1[:], xc[:, 0:128], w2h0[:], start=True, stop=False)
                    nc.tensor.matmul(p1[:], xc[:, 128:256], w2h1[:], start=False, stop=True)
                    p1sb = pool.tile([N1, WS], dt.bfloat16, tag="p1sb")
                    nc.scalar.copy(p1sb[:], p1[:])
                    # U = P1 * C  (all slices 128-wide, 64B-aligned -> 2x mode)
                    uv = pool.tile([N1, WS], dt.bfloat16, tag="uv")
                    t1 = pool.tile([N1, 128], dt.bfloat16, tag="t1")
                    t2 = pool.tile([N1, 128], dt.bfloat16, tag="t2")
                    t3 = pool.tile([N1, 128], dt.bfloat16, tag="t3")
                    t4 = pool.tile([N1, 128], dt.bfloat16, tag="t4")
                    nc.vector.tensor_mul(t1[:], p1sb[:, 0:128], xc[:, N2 : N2 + 128])
                    nc.vector.tensor_mul(t2[:], p1sb[:, 128:256], xc[:, N2 + 128 : N2 + 256])
                    nc.vector.tensor_sub(uv[:, 0:128], t1[:], t2[:])
                    if e % 2 == 0:
                        nc.vector.tensor_mul(t3[:], p1sb[:, 0:128], xc[:, N2 + 128 : N2 + 256])
                        nc.gpsimd.tensor_mul(t4[:], p1sb[:, 128:256], xc[:, N2 : N2 + 128])
                    else:
                        nc.gpsimd.tensor_mul(t3[:], p1sb[:, 0:128], xc[:, N2 + 128 : N2 + 256])
                        nc.gpsimd.tensor_mul(t4[:], p1sb[:, 128:256], xc[:, N2 : N2 + 128])
                    nc.vector.tensor_add(uv[:, 128:256], t3[:], t4[:])
                    nc.scalar.copy(uv[:, 256:257], p1sb[:, 256:257])
                    # stage 3: one accumulation group per PSUM bank (incl. DC col)
                    nc.tensor.matmul(pZA[:, 0:257], xc[:, N2 + WS : N2 + WS + 128], uv[:, 0:257],
                                     start=(e == 0), stop=(e == E - 1))
                    nc.tensor.matmul(pZB[:, 0:257], xc[:, N2 + WS + 128 : N2 + WS + 256], uv[:, 0:257],
                                     start=(e == 0), stop=(e == E - 1))
                # xf = Z (128, 258): [re-main | im-main | dc-re | dc-im]
                xf = bpool.tile([N1, WS], dt.bfloat16, tag="xf")
                pbsb = bpool.tile([N1, WS], dt.bfloat16, tag="pbsb")
                nc.scalar.copy(pbsb[:], pZB[:])
                nc.any.tensor_sub(xf[:, 0:128], pZA[:, 0:128], pbsb[:, 128:256])
                nc.any.tensor_add(xf[:, 128:256], pZA[:, 128:256], pbsb[:, 0:128])
                nc.any.tensor_copy(xf[:, 256:257], pZA[:, 256:257])
                nc.any.tensor_copy(xf[:, 257:258], pbsb[:, 256:257])
                # I1 transposed: G^T chunks = xf_chunk^T @ [E1c | E1s]
                pgA = pinv.tile([N1, 256], dt.float32, tag="pgA")
                pgB = pinv.tile([N1, 256], dt.float32, tag="pgB")
                psdc = pinv.tile([1, 128], dt.float32, tag="psdc")
                nc.tensor.matmul(pgA[:], xf[:, 0:128], e1cat[:, 0:256], start=True, stop=True)
                nc.tensor.matmul(pgB[:], xf[:, 128:256], e1cat[:, 0:256], start=True, stop=True)
                nc.tensor.matmul(psdc[:], xf[:, 256:257], e1cat[:, 0:128], start=True, stop=False)
                nc.tensor.matmul(psdc[:], xf[:, 257:258], e1cat[:, 256:384], start=False, stop=True)
                gbsb = bpool.tile([N1, 256], dt.bfloat16, tag="gbsb")
                nc.scalar.copy(gbsb[:], pgB[:])
                g_re = bpool.tile([N1, N1], dt.bfloat16, tag="gre")
                g_im = bpool.tile([N1, N1], dt.bfloat16, tag="gim")
                nc.any.tensor_sub(g_re[:], pgA[:, 0:128], gbsb[:, 128:256])
                nc.any.tensor_add(g_im[:], pgA[:, 128:256], gbsb[:, 0:128])
                # twiddle (transposed layout, d/N folded)
                gttre = bpool.tile([N1, N1], dt.bfloat16, tag="gttre")
                gttim = bpool.tile([N1, N1], dt.bfloat16, tag="gttim")
                i1 = bpool.tile([N1, N1], dt.bfloat16, tag="i1")
                i2 = bpool.tile([N1, N1], dt.bfloat16, tag="i2")
                nc.any.tensor_mul(i1[:], g_re[:], twtc[:])
                nc.any.tensor_mul(i2[:], g_im[:], twts[:])
                nc.any.tensor_sub(gttre[:], i1[:], i2[:])
                nc.any.tensor_mul(i1[:], g_re[:], twts[:])
                nc.any.tensor_mul(i2[:], g_im[:], twtc[:])
                nc.any.tensor_add(gttim[:], i1[:], i2[:])
                # DC row -> row 0 of gtdc (TW const folded into e2c1)
                nc.any.tensor_copy(gtdc[0:1, :], psdc[0:1, :])
                # I4
                for jc in range(2):
                    js = slice(128 * jc, 128 * jc + 128)
                    ps = pinv.tile([128, N1], dt.float32, tag="ps")
                    nc.tensor.matmul(ps[:], e2c0[:, js], gttre[:], start=True, stop=False)
                    nc.tensor.matmul(ps[:], e2sn0[:, js], gttim[:], start=False, stop=False)
                    nc.tensor.matmul(ps[:], e2c1[:, js], gtdc[:], start=False, stop=True)
                    y_sb = bpool.tile([128, N1], dt.float32, tag="ysb")
                    nc.scalar.copy(y_sb[:], ps[:])
                    nc.scalar.dma_start(out_d[b, js, :], y_sb[:])
    nc.compile()
    return nc


def kernel(time_latent, stems, targets, W_pos, b_pos):
    from concourse.bass_utils import run_bass_kernel_spmd

    z = np.einsum("bed,od->beo", time_latent.astype(F32), W_pos.astype(F32))
    z = z.reshape(B, E) + b_pos.reshape(1)[0]
    pos = 1.0 / (1.0 + np.exp(-z, dtype=F32))
    s = pos * np.float32(N)

    W2m, Tm, W1s, E1, TWt, E2m = _host_tables()
    k2m = np.arange(1, 129)
    kap = np.arange(N1) - 64

    # stems: (B,E,32768) -> (B,E,128,256) bf16, cols [n2<128 | n2>=128]
    x = stems.reshape(B, E, N2, N1).astype(BF16)
    x = x.reshape(B, E, 2, 128, N1).transpose(0, 1, 3, 2, 4).reshape(B, E, N1, N2)

    w2cat = np.concatenate(
        [W2m.real, W2m.imag, np.ones((N2, 1)), np.zeros((N2, 1))], 1)  # (256,258)

    nc = _build_graph()
    in_maps = []
    for c in range(NCORES):
        sl = slice(c * BC, (c + 1) * BC)
        s_c = s[sl].astype(np.float64)                          # (BC, E)
        A = np.exp(-2j * np.pi * s_c[..., None] * k2m / N)      # (BC,E,128)
        Bs = np.exp(-2j * np.pi * s_c[..., None] * kap / N1)    # (BC,E,128)
        C = Tm[None, None] * A[:, :, None, :]                   # (BC,E,128,128)
        M = W1s[None, None] * Bs[:, :, None, :]                 # (BC,E,128,128)
        zc = np.zeros(C.shape[:-1] + (1,))
        cm = np.concatenate([C.real, C.imag, zc, zc, M.real, M.imag], -1).astype(BF16)
        in_maps.append({
            "xmcm": np.ascontiguousarray(np.concatenate([x[sl], cm], -1)),
            "w2cat": w2cat.astype(BF16),
            "e1cat": np.concatenate([E1.real, E1.imag, -E1.imag], 1).astype(BF16),
            "twtc": TWt.real.astype(BF16),
            "twts": TWt.imag.astype(BF16),
            "e2c0": E2m.real.astype(BF16),
            "e2sn0": (-E2m.imag).astype(BF16),
            "e2c1": np.concatenate(
                [np.full((1, N2), 1.0 / (N * GSC)), np.zeros((127, N2))], 0).astype(BF16),
        })

    import os
    trace = bool(int(os.environ.get("ATHENA_TRACE", "0")))
    res = run_bass_kernel_spmd(nc, in_maps, core_ids=list(range(NCORES)), trace=trace)
    if trace:
        print(f"HW exec time: {res.exec_time_ns} ns")
    outs = [res.results[c]["out"].reshape(BC, N).astype(F32) for c in range(NCORES)]
    return np.concatenate(outs, 0).reshape(B, 1, N).astype(F32)


# revision 12
# speedup vs baseline: 1.6563x; 1.1263x over previous
"""AtomPlacementScheduler Trainium2 kernel (v5).

out[b] = sum_e irfft(rfft(stems[b,e]) * exp(-2i pi f s_be)),  s = sigmoid(TL@W+b)*N.

4-step FFT on the signed-frequency half grid k~ = k2 + 256*k1, k2 in [0,128],
k1 signed in [-64,63]; every conjugate pair of the real-signal spectrum is
covered exactly once (k2 in {0,128} self-paired weight 1, k2 in [1,127]
weight 2 + real part).  Column layout keeps the main k2 = 1..128 block
exactly 128 wide (64B-aligned slices -> DVE 2x mode); the DC column (k2=0,
where all phases are 1) rides along via 1-column matmuls.  Per-event device
work: 2 stage-1 matmuls, 1 PSUM->SBUF copy, 6 x 128-wide elementwise ops
(split DVE/GpSimd, alternating), 2+2 stage-3 matmuls accumulating the event
sum in PSUM.  C = T*A_e and M = W1*B_e are host-precomputed, one fused bf16
DMA per event.  The inverse runs transposed (stationary = Z chunks) so no
DMA transposes are needed; it is exact (no host correction).

Self-contained: hardcodes shapes B=64, E=16, N=32768, n_cores=8.
"""
import numpy as np
import ml_dtypes

N = 32768
N1 = 128
N2 = 256
E = 16
B = 64
NCORES = 8
BC = B // NCORES
F32 = np.float32
BF16 = ml_dtypes.bfloat16
GSC = np.float32(1.0 / 16.0)
WMAIN = 256          # [re(k2=1..128) | im(k2=1..128)]
WS = 258             # + [dc-re | dc-im]
CMW = WS + 256       # c section + m section


def _host_tables():
    n1 = np.arange(N1)
    n2 = np.arange(N2)
    k2m = np.arange(1, 129)
    kap = np.arange(N1) - 64
    W2m = np.exp(-2j * np.pi * np.outer(n2, k2m) / N2)       # (256,128)
    Tm = np.exp(-2j * np.pi * np.outer(n1, k2m) / N)         # (128,128)
    W1s = np.exp(-2j * np.pi * np.outer(n1, kap) / N1)       # (128,128)
    E1 = np.exp(+2j * np.pi * np.outer(kap, n1) / N1) * GSC  # (j,n1)
    d = np.where(k2m == 128, 1.0, 2.0)
    TWt = np.exp(+2j * np.pi * np.outer(k2m, n1) / N) * (d[:, None] / (N * GSC))
    E2m = np.exp(+2j * np.pi * np.outer(k2m, n2) / N2)       # (128,256)
    return W2m, Tm, W1s, E1, TWt, E2m


def _build_graph():
    import concourse.bass as bass
    import concourse.mybir as mybir
    import concourse.tile as tile
    from concourse import bacc

    dt = mybir.dt
    nc = bacc.Bacc("TRN2", target_bir_lowering=False, debug=False, num_devices=NCORES)

    xc_d = nc.dram_tensor("xmcm", [BC, E, N1, 768], dt.bfloat16, kind="ExternalInput")
    w2_d = nc.dram_tensor("w2cat", [N2, WS], dt.bfloat16, kind="ExternalInput")
    e1_d = nc.dram_tensor("e1cat", [N1, 384], dt.bfloat16, kind="ExternalInput")
    twc_d = nc.dram_tensor("twtc", [N1, N1], dt.bfloat16, kind="ExternalInput")
    tws_d = nc.dram_tensor("twts", [N1, N1], dt.bfloat16, kind="ExternalInput")
    e2c0_d = nc.dram_tensor("e2c0", [128, N2], dt.bfloat16, kind="ExternalInput")
    e2sn0_d = nc.dram_tensor("e2sn0", [128, N2], dt.bfloat16, kind="ExternalInput")
    e2c1_d = nc.dram_tensor("e2c1", [128, N2], dt.bfloat16, kind="ExternalInput")
    out_d = nc.dram_tensor("out", [BC, N2, N1], dt.float32, kind="ExternalOutput")

    with tile.TileContext(nc) as tc:
        with (
            tc.tile_pool(name="const", bufs=1) as cpool,
            tc.tile_pool(name="work", bufs=6) as pool,
            tc.tile_pool(name="binv", bufs=2) as bpool,
            tc.tile_pool(name="psum", bufs=2, space="PSUM") as psum,
            tc.tile_pool(name="psacc", bufs=1, space="PSUM") as psacc,
            tc.tile_pool(name="pinv", bufs=1, space="PSUM") as pinv,
        ):
            w2h0 = cpool.tile([128, WS], dt.bfloat16, tag="w2h0")
            w2h1 = cpool.tile([128, WS], dt.bfloat16, tag="w2h1")
            nc.sync.dma_start(w2h0[:], w2_d[0:128, :])
            nc.sync.dma_start(w2h1[:], w2_d[128:256, :])
            e1cat = cpool.tile([N1, 384], dt.bfloat16, tag="e1cat")
            nc.sync.dma_start(e1cat[:], e1_d[:])
            twtc = cpool.tile([N1, N1], dt.bfloat16, tag="twtc")
            twts = cpool.tile([N1, N1], dt.bfloat16, tag="twts")
            nc.sync.dma_start(twtc[:], twc_d[:])
            nc.sync.dma_start(twts[:], tws_d[:])
            e2c0 = cpool.tile([128, N2], dt.bfloat16, tag="e2c0")
            e2sn0 = cpool.tile([128, N2], dt.bfloat16, tag="e2sn0")
            e2c1 = cpool.tile([128, N2], dt.bfloat16, tag="e2c1")
            nc.sync.dma_start(e2c0[:], e2c0_d[:])
            nc.sync.dma_start(e2sn0[:], e2sn0_d[:])
            nc.sync.dma_start(e2c1[:], e2c1_d[:])
            gtdc = cpool.tile([128, N1], dt.bfloat16, tag="gtdc")
            nc.vector.memset(gtdc[:], 0.0)

            for b in range(BC):
                pZA = psacc.tile([N1, WS], dt.float32, tag="pZA")
                pZB = psacc.tile([N1, WS], dt.float32, tag="pZB")
                for e in range(E):
                    xc = pool.tile([128, 768], dt.bfloat16, tag="xc")
                    nc.sync.dma_start(xc[:], xc_d[b, e])
                    p1 = psum.tile([N1, WS], dt.float32, tag="p1")
                    nc.tensor.matmul(p1[:], xc[:, 0:128], w2h0[:], start=True, stop=False)
                    nc.tensor.matmul(p1[:], xc[:, 128:256], w2h1[:], start=False, stop=True)
                    p1sb = pool.tile([N1, 320], dt.bfloat16, tag="p1sb")
                    nc.scalar.copy(p1sb[:, 0:WS], p1[:])
                    # U = P1 * C  (all slices 128-wide, 64B-aligned -> 2x mode)
                    uv = pool.tile([N1, 320], dt.bfloat16, tag="uv")
                    t1 = pool.tile([N1, 128], dt.bfloat16, tag="t1")
                    t2 = pool.tile([N1, 128], dt.bfloat16, tag="t2")
                    t3 = pool.tile([N1, 128], dt.bfloat16, tag="t3")
                    t4 = pool.tile([N1, 128], dt.bfloat16, tag="t4")
                    nc.vector.tensor_mul(t1[:], p1sb[:, 0:128], xc[:, 256:384])
                    nc.vector.tensor_mul(t2[:], p1sb[:, 128:256], xc[:, 384:512])
                    nc.vector.tensor_sub(uv[:, 0:128], t1[:], t2[:])
                    if e % 2 == 0:
                        nc.vector.tensor_mul(t3[:], p1sb[:, 0:128], xc[:, 384:512])
                        nc.gpsimd.tensor_mul(t4[:], p1sb[:, 128:256], xc[:, 256:384])
                    else:
                        nc.gpsimd.tensor_mul(t3[:], p1sb[:, 0:128], xc[:, 384:512])
                        nc.gpsimd.tensor_mul(t4[:], p1sb[:, 128:256], xc[:, 256:384])
                    nc.vector.tensor_add(uv[:, 128:256], t3[:], t4[:])
                    nc.vector.tensor_copy(uv[:, 256:257], p1sb[:, 256:257])
                    # stage 3: one accumulation group per PSUM bank (incl. DC col)
                    nc.tensor.matmul(pZA[:, 0:257], xc[:, 512:640], uv[:, 0:257],
                                     start=(e == 0), stop=(e == E - 1))
                    nc.tensor.matmul(pZB[:, 0:257], xc[:, 640:768], uv[:, 0:257],
                                     start=(e == 0), stop=(e == E - 1))
                # xf = Z (128, 258): [re-main | im-main | dc-re | dc-im]
                xf = bpool.tile([N1, WS], dt.bfloat16, tag="xf")
                pbsb = bpool.tile([N1, WS], dt.bfloat16, tag="pbsb")
                nc.scalar.copy(pbsb[:], pZB[:])
                nc.any.tensor_sub(xf[:, 0:128], pZA[:, 0:128], pbsb[:, 128:256])
                nc.any.tensor_add(xf[:, 128:256], pZA[:, 128:256], pbsb[:, 0:128])
                nc.any.tensor_copy(xf[:, 256:257], pZA[:, 256:257])
                nc.any.tensor_copy(xf[:, 257:258], pbsb[:, 256:257])
                # I1 transposed: G^T chunks = xf_chunk^T @ [E1c | E1s]
                pgA = pinv.tile([N1, 256], dt.float32, tag="pgA")
                pgB = pinv.tile([N1, 256], dt.float32, tag="pgB")
                psdc = pinv.tile([1, 128], dt.float32, tag="psdc")
                nc.tensor.matmul(pgA[:], xf[:, 0:128], e1cat[:, 0:256], start=True, stop=True)
                nc.tensor.matmul(pgB[:], xf[:, 128:256], e1cat[:, 0:256], start=True, stop=True)
                nc.tensor.matmul(psdc[:], xf[:, 256:257], e1cat[:, 0:128], start=True, stop=False)
                nc.tensor.matmul(psdc[:], xf[:, 257:258], e1cat[:, 256:384], start=False, stop=True)
                gbsb = bpool.tile([N1, 256], dt.bfloat16, tag="gbsb")
                nc.scalar.copy(gbsb[:], pgB[:])
                g_re = bpool.tile([N1, N1], dt.bfloat16, tag="gre")
                g_im = bpool.tile([N1, N1], dt.bfloat16, tag="gim")
                nc.any.tensor_sub(g_re[:], pgA[:, 0:128], gbsb[:, 128:256])
                nc.any.tensor_add(g_im[:], pgA[:, 128:256], gbsb[:, 0:128])
                # twiddle (transposed layout, d/N folded)
                gttre = bpool.tile([N1, N1], dt.bfloat16, tag="gttre")
                gttim = bpool.tile([N1, N1], dt.bfloat16, tag="gttim")
                i1 = bpool.tile([N1, N1], dt.bfloat16, tag="i1")
                i2 = bpool.tile([N1, N1], dt.bfloat16, tag="i2")
                nc.any.tensor_mul(i1[:], g_re[:], twtc[:])
                nc.any.tensor_mul(i2[:], g_im[:], twts[:])
                nc.any.tensor_sub(gttre[:], i1[:], i2[:])
                nc.any.tensor_mul(i1[:], g_re[:], twts[:])
                nc.any.tensor_mul(i2[:], g_im[:], twtc[:])
                nc.any.tensor_add(gttim[:], i1[:], i2[:])
                # DC row -> row 0 of gtdc (TW const folded into e2c1)
                nc.any.tensor_copy(gtdc[0:1, :], psdc[0:1, :])
                # I4
                for jc in range(2):
                    js = slice(128 * jc, 128 * jc + 128)
                    ps = pinv.tile([128, N1], dt.float32, tag="ps")
                    nc.tensor.matmul(ps[:], e2c0[:, js], gttre[:], start=True, stop=False)
                    nc.tensor.matmul(ps[:], e2sn0[:, js], gttim[:], start=False, stop=False)
                    nc.tensor.matmul(ps[:], e2c1[:, js], gtdc[:], start=False, stop=True)
                    y_sb = bpool.tile([128, N1], dt.float32, tag="ysb")
                    nc.scalar.copy(y_sb[:], ps[:])
                    nc.scalar.dma_start(out_d[b, js, :], y_sb[:])
    nc.compile()
    return nc


def kernel(time_latent, stems, targets, W_pos, b_pos):
    from concourse.bass_utils import run_bass_kernel_spmd

    z = np.einsum("bed,od->beo", time_latent.astype(F32), W_pos.astype(F32))
    z = z.reshape(B, E) + b_pos.reshape(1)[0]
    pos = 1.0 / (1.0 + np.exp(-z, dtype=F32))
    s = pos * np.float32(N)

    W2m, Tm, W1s, E1, TWt, E2m = _host_tables()
    k2m = np.arange(1, 129)
    kap = np.arange(N1) - 64

    # stems: (B,E,32768) -> (B,E,128,256) bf16, cols [n2<128 | n2>=128]
    x = stems.reshape(B, E, N2, N1).astype(BF16)
    x = x.reshape(B, E, 2, 128, N1).transpose(0, 1, 3, 2, 4).reshape(B, E, N1, N2)

    w2cat = np.concatenate(
        [W2m.real, W2m.imag, np.ones((N2, 1)), np.zeros((N2, 1))], 1)  # (256,258)

    nc = _build_graph()
    in_maps = []
    for c in range(NCORES):
        sl = slice(c * BC, (c + 1) * BC)
        s_c = s[sl].astype(np.float64)                          # (BC, E)
        A = np.exp(-2j * np.pi * s_c[..., None] * k2m / N)      # (BC,E,128)
        Bs = np.exp(-2j * np.pi * s_c[..., None] * kap / N1)    # (BC,E,128)
        C = Tm[None, None] * A[:, :, None, :]                   # (BC,E,128,128)
        M = W1s[None, None] * Bs[:, :, None, :]                 # (BC,E,128,128)
        cm = np.concatenate([C.real, C.imag, M.real, M.imag], -1).astype(BF16)
        in_maps.append({
            "xmcm": np.ascontiguousarray(np.concatenate([x[sl], cm], -1)),
            "w2cat": w2cat.astype(BF16),
            "e1cat": np.concatenate([E1.real, E1.imag, -E1.imag], 1).astype(BF16),
            "twtc": TWt.real.astype(BF16),
            "twts": TWt.imag.astype(BF16),
            "e2c0": E2m.real.astype(BF16),
            "e2sn0": (-E2m.imag).astype(BF16),
            "e2c1": np.concatenate(
                [np.full((1, N2), 1.0 / (N * GSC)), np.zeros((127, N2))], 0).astype(BF16),
        })

    import os
    trace = bool(int(os.environ.get("ATHENA_TRACE", "0")))
    res = run_bass_kernel_spmd(nc, in_maps, core_ids=list(range(NCORES)), trace=trace)
    if trace:
        print(f"HW exec time: {res.exec_time_ns} ns")
    outs = [res.results[c]["out"].reshape(BC, N).astype(F32) for c in range(NCORES)]
    return np.concatenate(outs, 0).reshape(B, 1, N).astype(F32)


# revision 13
# speedup vs baseline: 1.6725x; 1.0098x over previous
"""AtomPlacementScheduler Trainium2 kernel (v5).

out[b] = sum_e irfft(rfft(stems[b,e]) * exp(-2i pi f s_be)),  s = sigmoid(TL@W+b)*N.

4-step FFT on the signed-frequency half grid k~ = k2 + 256*k1, k2 in [0,128],
k1 signed in [-64,63]; every conjugate pair of the real-signal spectrum is
covered exactly once (k2 in {0,128} self-paired weight 1, k2 in [1,127]
weight 2 + real part).  Column layout keeps the main k2 = 1..128 block
exactly 128 wide (64B-aligned slices -> DVE 2x mode); the DC column (k2=0,
where all phases are 1) rides along via 1-column matmuls.  Per-event device
work: 2 stage-1 matmuls, 1 PSUM->SBUF copy, 6 x 128-wide elementwise ops
(split DVE/GpSimd, alternating), 2+2 stage-3 matmuls accumulating the event
sum in PSUM.  C = T*A_e and M = W1*B_e are host-precomputed, one fused bf16
DMA per event.  The inverse runs transposed (stationary = Z chunks) so no
DMA transposes are needed; it is exact (no host correction).

Self-contained: hardcodes shapes B=64, E=16, N=32768, n_cores=8.
"""
import numpy as np
import ml_dtypes

N = 32768
N1 = 128
N2 = 256
E = 16
B = 64
NCORES = 8
BC = B // NCORES
F32 = np.float32
BF16 = ml_dtypes.bfloat16
GSC = np.float32(1.0 / 16.0)
WMAIN = 256          # [re(k2=1..128) | im(k2=1..128)]
WS = 258             # + [dc-re | dc-im]
CMW = WS + 256       # c section + m section


def _host_tables():
    n1 = np.arange(N1)
    n2 = np.arange(N2)
    k2m = np.arange(1, 129)
    kap = np.arange(N1) - 64
    W2m = np.exp(-2j * np.pi * np.outer(n2, k2m) / N2)       # (256,128)
    Tm = np.exp(-2j * np.pi * np.outer(n1, k2m) / N)         # (128,128)
    W1s = np.exp(-2j * np.pi * np.outer(n1, kap) / N1)       # (128,128)
    E1 = np.exp(+2j * np.pi * np.outer(kap, n1) / N1) * GSC  # (j,n1)
    d = np.where(k2m == 128, 1.0, 2.0)
    TWt = np.exp(+2j * np.pi * np.outer(k2m, n1) / N) * (d[:, None] / (N * GSC))
    E2m = np.exp(+2j * np.pi * np.outer(k2m, n2) / N2)       # (128,256)
    return W2m, Tm, W1s, E1, TWt, E2m


def _build_graph():
    import concourse.bass as bass
    import concourse.mybir as mybir
    import concourse.tile as tile
    from concourse import bacc

    dt = mybir.dt
    nc = bacc.Bacc("TRN2", target_bir_lowering=False, debug=False, num_devices=NCORES)

    xc_d = nc.dram_tensor("xmcm", [BC, E, N1, 1024], dt.bfloat16, kind="ExternalInput")
    w2_d = nc.dram_tensor("w2cat", [N2, WS], dt.bfloat16, kind="ExternalInput")
    e1_d = nc.dram_tensor("e1cat", [N1, 384], dt.bfloat16, kind="ExternalInput")
    twc_d = nc.dram_tensor("twtc", [N1, N1], dt.bfloat16, kind="ExternalInput")
    tws_d = nc.dram_tensor("twts", [N1, N1], dt.bfloat16, kind="ExternalInput")
    e2c0_d = nc.dram_tensor("e2c0", [128, N2], dt.bfloat16, kind="ExternalInput")
    e2sn0_d = nc.dram_tensor("e2sn0", [128, N2], dt.bfloat16, kind="ExternalInput")
    e2c1_d = nc.dram_tensor("e2c1", [128, N2], dt.bfloat16, kind="ExternalInput")
    out_d = nc.dram_tensor("out", [BC, N2, N1], dt.float32, kind="ExternalOutput")

    with tile.TileContext(nc) as tc:
        with (
            tc.tile_pool(name="const", bufs=1) as cpool,
            tc.tile_pool(name="work", bufs=6) as pool,
            tc.tile_pool(name="binv", bufs=2) as bpool,
            tc.tile_pool(name="psum", bufs=2, space="PSUM") as psum,
            tc.tile_pool(name="psacc", bufs=1, space="PSUM") as psacc,
            tc.tile_pool(name="pinv", bufs=1, space="PSUM") as pinv,
        ):
            w2h0 = cpool.tile([128, WS], dt.bfloat16, tag="w2h0")
            w2h1 = cpool.tile([128, WS], dt.bfloat16, tag="w2h1")
            nc.sync.dma_start(w2h0[:], w2_d[0:128, :])
            nc.sync.dma_start(w2h1[:], w2_d[128:256, :])
            e1cat = cpool.tile([N1, 384], dt.bfloat16, tag="e1cat")
            nc.sync.dma_start(e1cat[:], e1_d[:])
            twtc = cpool.tile([N1, N1], dt.bfloat16, tag="twtc")
            twts = cpool.tile([N1, N1], dt.bfloat16, tag="twts")
            nc.sync.dma_start(twtc[:], twc_d[:])
            nc.sync.dma_start(twts[:], tws_d[:])
            e2c0 = cpool.tile([128, N2], dt.bfloat16, tag="e2c0")
            e2sn0 = cpool.tile([128, N2], dt.bfloat16, tag="e2sn0")
            e2c1 = cpool.tile([128, N2], dt.bfloat16, tag="e2c1")
            nc.sync.dma_start(e2c0[:], e2c0_d[:])
            nc.sync.dma_start(e2sn0[:], e2sn0_d[:])
            nc.sync.dma_start(e2c1[:], e2c1_d[:])
            gtdc = cpool.tile([128, N1], dt.bfloat16, tag="gtdc")
            nc.vector.memset(gtdc[:], 0.0)

            for b in range(BC):
                pZA = psacc.tile([N1, WS], dt.float32, tag="pZA")
                pZB = psacc.tile([N1, WS], dt.float32, tag="pZB")
                for e in range(E):
                    xc = pool.tile([128, 1024], dt.bfloat16, tag="xc")
                    nc.sync.dma_start(xc[:], xc_d[b, e])
                    p1 = psum.tile([N1, WS], dt.float32, tag="p1")
                    nc.tensor.matmul(p1[:], xc[:, 0:128], w2h0[:], start=True, stop=False)
                    nc.tensor.matmul(p1[:], xc[:, 128:256], w2h1[:], start=False, stop=True)
                    p1sb = pool.tile([N1, 320], dt.bfloat16, tag="p1sb")
                    nc.scalar.copy(p1sb[:, 0:WS], p1[:])
                    # U = P1 * C via 3 fused ops: [t1|t3] = p1re * [Cre|Cim],
                    # [-t2|t4] = p1im * [-Cim|Cre], uv = sum  (repeat-AP operands)
                    uv = pool.tile([N1, 320], dt.bfloat16, tag="uv")
                    t13 = pool.tile([N1, 256], dt.bfloat16, tag="t13")
                    t24 = pool.tile([N1, 256], dt.bfloat16, tag="t24")
                    p1re = p1sb[:, 0:128].unsqueeze(1).broadcast_to([128, 2, 128])
                    p1im = p1sb[:, 128:256].unsqueeze(1).broadcast_to([128, 2, 128])
                    nc.vector.tensor_mul(t13[:], p1re, xc[:, 256:512])
                    if e % 2 == 0:
                        nc.gpsimd.tensor_mul(t24[:], p1im, xc[:, 512:768])
                    else:
                        nc.vector.tensor_mul(t24[:], p1im, xc[:, 512:768])
                    nc.vector.tensor_add(uv[:, 0:256], t13[:], t24[:])
                    nc.vector.tensor_copy(uv[:, 256:257], p1sb[:, 256:257])
                    # stage 3: one accumulation group per PSUM bank (incl. DC col)
                    nc.tensor.matmul(pZA[:, 0:257], xc[:, 768:896], uv[:, 0:257],
                                     start=(e == 0), stop=(e == E - 1))
                    nc.tensor.matmul(pZB[:, 0:257], xc[:, 896:1024], uv[:, 0:257],
                                     start=(e == 0), stop=(e == E - 1))
                # xf = Z (128, 258): [re-main | im-main | dc-re | dc-im]
                xf = bpool.tile([N1, WS], dt.bfloat16, tag="xf")
                pbsb = bpool.tile([N1, WS], dt.bfloat16, tag="pbsb")
                nc.scalar.copy(pbsb[:], pZB[:])
                nc.any.tensor_sub(xf[:, 0:128], pZA[:, 0:128], pbsb[:, 128:256])
                nc.any.tensor_add(xf[:, 128:256], pZA[:, 128:256], pbsb[:, 0:128])
                nc.any.tensor_copy(xf[:, 256:257], pZA[:, 256:257])
                nc.any.tensor_copy(xf[:, 257:258], pbsb[:, 256:257])
                # I1 transposed: G^T chunks = xf_chunk^T @ [E1c | E1s]
                pgA = pinv.tile([N1, 256], dt.float32, tag="pgA")
                pgB = pinv.tile([N1, 256], dt.float32, tag="pgB")
                psdc = pinv.tile([1, 128], dt.float32, tag="psdc")
                nc.tensor.matmul(pgA[:], xf[:, 0:128], e1cat[:, 0:256], start=True, stop=True)
                nc.tensor.matmul(pgB[:], xf[:, 128:256], e1cat[:, 0:256], start=True, stop=True)
                nc.tensor.matmul(psdc[:], xf[:, 256:257], e1cat[:, 0:128], start=True, stop=False)
                nc.tensor.matmul(psdc[:], xf[:, 257:258], e1cat[:, 256:384], start=False, stop=True)
                gbsb = bpool.tile([N1, 256], dt.bfloat16, tag="gbsb")
                nc.scalar.copy(gbsb[:], pgB[:])
                g_re = bpool.tile([N1, N1], dt.bfloat16, tag="gre")
                g_im = bpool.tile([N1, N1], dt.bfloat16, tag="gim")
                nc.any.tensor_sub(g_re[:], pgA[:, 0:128], gbsb[:, 128:256])
                nc.any.tensor_add(g_im[:], pgA[:, 128:256], gbsb[:, 0:128])
                # twiddle (transposed layout, d/N folded)
                gttre = bpool.tile([N1, N1], dt.bfloat16, tag="gttre")
                gttim = bpool.tile([N1, N1], dt.bfloat16, tag="gttim")
                i1 = bpool.tile([N1, N1], dt.bfloat16, tag="i1")
                i2 = bpool.tile([N1, N1], dt.bfloat16, tag="i2")
                nc.any.tensor_mul(i1[:], g_re[:], twtc[:])
                nc.any.tensor_mul(i2[:], g_im[:], twts[:])
                nc.any.tensor_sub(gttre[:], i1[:], i2[:])
                nc.any.tensor_mul(i1[:], g_re[:], twts[:])
                nc.any.tensor_mul(i2[:], g_im[:], twtc[:])
                nc.any.tensor_add(gttim[:], i1[:], i2[:])
                # DC row -> row 0 of gtdc (TW const folded into e2c1)
                nc.any.tensor_copy(gtdc[0:1, :], psdc[0:1, :])
                # I4
                for jc in range(2):
                    js = slice(128 * jc, 128 * jc + 128)
                    ps = pinv.tile([128, N1], dt.float32, tag="ps")
                    nc.tensor.matmul(ps[:], e2c0[:, js], gttre[:], start=True, stop=False)
                    nc.tensor.matmul(ps[:], e2sn0[:, js], gttim[:], start=False, stop=False)
                    nc.tensor.matmul(ps[:], e2c1[:, js], gtdc[:], start=False, stop=True)
                    y_sb = bpool.tile([128, N1], dt.float32, tag="ysb")
                    nc.scalar.copy(y_sb[:], ps[:])
                    nc.scalar.dma_start(out_d[b, js, :], y_sb[:])
    nc.compile()
    return nc


def kernel(time_latent, stems, targets, W_pos, b_pos):
    from concourse.bass_utils import run_bass_kernel_spmd

    z = np.einsum("bed,od->beo", time_latent.astype(F32), W_pos.astype(F32))
    z = z.reshape(B, E) + b_pos.reshape(1)[0]
    pos = 1.0 / (1.0 + np.exp(-z, dtype=F32))
    s = pos * np.float32(N)

    W2m, Tm, W1s, E1, TWt, E2m = _host_tables()
    k2m = np.arange(1, 129)
    kap = np.arange(N1) - 64

    # stems: (B,E,32768) -> (B,E,128,256) bf16, cols [n2<128 | n2>=128]
    x = stems.reshape(B, E, N2, N1).astype(BF16)
    x = x.reshape(B, E, 2, 128, N1).transpose(0, 1, 3, 2, 4).reshape(B, E, N1, N2)

    w2cat = np.concatenate(
        [W2m.real, W2m.imag, np.ones((N2, 1)), np.zeros((N2, 1))], 1)  # (256,258)

    nc = _build_graph()
    in_maps = []
    for c in range(NCORES):
        sl = slice(c * BC, (c + 1) * BC)
        s_c = s[sl].astype(np.float64)                          # (BC, E)
        A = np.exp(-2j * np.pi * s_c[..., None] * k2m / N)      # (BC,E,128)
        Bs = np.exp(-2j * np.pi * s_c[..., None] * kap / N1)    # (BC,E,128)
        C = Tm[None, None] * A[:, :, None, :]                   # (BC,E,128,128)
        M = W1s[None, None] * Bs[:, :, None, :]                 # (BC,E,128,128)
        cm = np.concatenate([C.real, C.imag, -C.imag, C.real,
                             M.real, M.imag], -1).astype(BF16)
        in_maps.append({
            "xmcm": np.ascontiguousarray(np.concatenate([x[sl], cm], -1)),
            "w2cat": w2cat.astype(BF16),
            "e1cat": np.concatenate([E1.real, E1.imag, -E1.imag], 1).astype(BF16),
            "twtc": TWt.real.astype(BF16),
            "twts": TWt.imag.astype(BF16),
            "e2c0": E2m.real.astype(BF16),
            "e2sn0": (-E2m.imag).astype(BF16),
            "e2c1": np.concatenate(
                [np.full((1, N2), 1.0 / (N * GSC)), np.zeros((127, N2))], 0).astype(BF16),
        })

    import os
    trace = bool(int(os.environ.get("ATHENA_TRACE", "0")))
    res = run_bass_kernel_spmd(nc, in_maps, core_ids=list(range(NCORES)), trace=trace)
    if trace:
        print(f"HW exec time: {res.exec_time_ns} ns")
    outs = [res.results[c]["out"].reshape(BC, N).astype(F32) for c in range(NCORES)]
    return np.concatenate(outs, 0).reshape(B, 1, N).astype(F32)
